# revision 2
# baseline (speedup 1.0000x reference)
"""JukeboxAttention Trainium2 kernel, v2.

Data-parallel: 32 independent attention blocks sharded 4-per-core across 8
NeuronCores. All weight/activation DMAs are fully contiguous per partition
(host pre-lays-out x^T and both weight matrices in SBUF-native order), and
matmuls run in bf16 (fp32 PSUM accumulation). Per block: qkv projection per
head, causal block attention in [k, q] layout, fused 16-head softmax
denominators in one PSUM tile, then out = ctx @ c_proj_w + b.
"""

import sys

sys.path.insert(0, "/opt/trn_rl_repo")

import numpy as np

B, L, E = 2, 8192, 2048
HEADS, HD = 16, 128
BLOCKS, BC = 16, 512
SCALE2 = float(HD) ** -0.5  # (hd^-0.25)^2, folded onto q
NCORES = 8
BPC = B * BLOCKS // NCORES  # blocks per core = 4
T = BPC * BC  # tokens per core = 2048
ET = E // 128  # 16 contraction tiles
NEG = -1e9


def _build_nc(reps=1):
    import concourse.bass as bass  # noqa: F401
    from concourse import bacc, mybir, tile

    f32 = mybir.dt.float32
    bf16 = mybir.dt.bfloat16
    R = mybir.dt.float32r
    Act = mybir.ActivationFunctionType

    nc = bacc.Bacc("TRN2", target_bir_lowering=False, debug=False)

    xsT = nc.dram_tensor("xsT", [E, T], bf16, kind="ExternalInput").ap()
    waq = nc.dram_tensor("waq", [128, HEADS, 3, ET, 128], bf16,
                         kind="ExternalInput").ap()
    cab = nc.dram_tensor("cab", [128, HEADS * 3], f32, kind="ExternalInput").ap()
    wpr = nc.dram_tensor("wpr", [128, 4, HEADS, 512], bf16,
                         kind="ExternalInput").ap()
    cpb = nc.dram_tensor("cpb", [E], f32, kind="ExternalInput").ap()
    mneg = nc.dram_tensor("mneg", [128, 4, BC], f32, kind="ExternalInput").ap()
    idnt = nc.dram_tensor("idnt", [128, 128], bf16, kind="ExternalInput").ap()
    osel = nc.dram_tensor("osel", [128, HEADS, HEADS], bf16,
                          kind="ExternalInput").ap()
    sbc = nc.dram_tensor("sbc", [HEADS, HEADS, 128], f32,
                         kind="ExternalInput").ap()
    out = nc.dram_tensor("out", [T, E], f32, kind="ExternalOutput").ap()

    with tile.TileContext(nc) as tc:
        with (
            tc.tile_pool(name="const", bufs=1) as const,
            tc.tile_pool(name="xt", bufs=1) as xtp,
            tc.tile_pool(name="ctxt", bufs=1) as ctxp,
            tc.tile_pool(name="wa", bufs=3) as wap,
            tc.tile_pool(name="wp", bufs=1) as wpp,
            tc.tile_pool(name="qkv", bufs=4) as qkvp,
            tc.tile_pool(name="vh", bufs=2) as vhp,
            tc.tile_pool(name="probs", bufs=6) as prp,
            tc.tile_pool(name="sums", bufs=2) as sup,
            tc.tile_pool(name="rbc", bufs=2) as rbp,
            tc.tile_pool(name="osb", bufs=2) as outp,
            tc.tile_pool(name="psb", bufs=5, space="PSUM") as psb,
            tc.tile_pool(name="psv", bufs=1, space="PSUM") as psv,
            tc.tile_pool(name="pssum", bufs=1, space="PSUM") as pssum_p,
            tc.tile_pool(name="psbc", bufs=1, space="PSUM") as psbc_p,
        ):
            # ---- constants ----
            ident_sb = const.tile([128, 128], bf16, tag="ident")
            nc.sync.dma_start(out=ident_sb, in_=idnt)
            mneg_sb = const.tile([128, 4, BC], f32, tag="mneg")
            nc.sync.dma_start(out=mneg_sb, in_=mneg)
            cab_sb = const.tile([128, HEADS * 3], f32, tag="cab")
            nc.sync.dma_start(out=cab_sb, in_=cab)
            osel_sb = const.tile([128, HEADS, HEADS], bf16, tag="osel")
            nc.sync.dma_start(out=osel_sb, in_=osel)
            sbc_sb = const.tile([HEADS, HEADS, 128], R, tag="sbc")
            nc.sync.dma_start(out=sbc_sb, in_=sbc.bitcast(R))
            pbias = const.tile([128, E], f32, tag="pbias")
            pb_ap = bass.AP(tensor=cpb.tensor, offset=cpb.offset,
                            ap=[[0, 128], [1, E]])
            nc.gpsimd.dma_start(out=pbias, in_=pb_ap)

            Xt = xtp.tile([128, ET, T], bf16, tag="xt")
            for et in range(ET):
                nc.sync.dma_start(out=Xt[:, et, :],
                                  in_=xsT[et * 128:(et + 1) * 128, :])
            ctxT = ctxp.tile([128, HEADS, T], bf16, tag="ctxt")

            for rep in range(reps):
                for blk in range(BPC):
                    c0 = blk * BC
                    ps_sums = pssum_p.tile([HEADS, BC], f32, tag="pssum")
                    for h in range(HEADS):
                        qkv_sb = []
                        for c in range(3):
                            wac = wap.tile([128, ET, 128], bf16, tag="wa")
                            nc.sync.dma_start(out=wac, in_=waq[:, h, c])
                            ps = psb.tile([128, BC], f32, tag="ps",
                                          name=f"ps_qkv{c}")
                            for et in range(ET):
                                nc.tensor.matmul(ps, lhsT=wac[:, et, :],
                                                 rhs=Xt[:, et, c0:c0 + BC],
                                                 start=(et == 0),
                                                 stop=(et == ET - 1))
                            sb = qkvp.tile([128, BC], bf16, tag="qkv",
                                           name=f"qkv{c}")
                            fi = h * 3 + c
                            nc.scalar.activation(
                                sb, ps, Act.Identity,
                                bias=cab_sb[:, fi:fi + 1],
                                scale=(SCALE2 if c == 0 else 1.0))
                            qkv_sb.append(sb)
                        q_sb, k_sb, v_sb = qkv_sb

                        # v -> natural [token, hd] layout via PE transpose
                        pv = psv.tile([128, 4, 128], bf16, tag="psbf",
                                      name="pv")
                        with nc.allow_low_precision(
                                reason="bf16 pass-through PE transpose"):
                            for kt in range(4):
                                nc.tensor.transpose(
                                    pv[:, kt, :],
                                    v_sb[:, kt * 128:(kt + 1) * 128],
                                    ident_sb)
                        v_head = vhp.tile([128, 4, 128], bf16, tag="vh")
                        nc.vector.tensor_copy(v_head, pv)

                        # scores^T [k, q]: +NEG mask, exp
                        pbs = []
                        for kt in range(4):
                            ps_s = psb.tile([128, BC], f32, tag="ps",
                                            name=f"ps_s{kt}")
                            nc.tensor.matmul(
                                ps_s, lhsT=k_sb[:, kt * 128:(kt + 1) * 128],
                                rhs=q_sb, start=True, stop=True)
                            nc.vector.tensor_add(ps_s, ps_s, mneg_sb[:, kt, :])
                            pb = prp.tile([128, BC], bf16, tag="pb")
                            nc.scalar.activation(pb, ps_s, Act.Exp)
                            pbs.append(pb)

                        # denominators for all heads into one [16, BC] psum
                        for kt in range(4):
                            nc.tensor.matmul(
                                ps_sums, lhsT=osel_sb[:, h, :], rhs=pbs[kt],
                                start=(h == 0 and kt == 0),
                                stop=(h == HEADS - 1 and kt == 3))

                        # ctx^T accumulate
                        ps_c = psb.tile([128, BC], f32, tag="ps", name="ps_c")
                        for kt in range(4):
                            nc.tensor.matmul(ps_c, lhsT=v_head[:, kt, :],
                                             rhs=pbs[kt], start=(kt == 0),
                                             stop=(kt == 3))
                        nc.scalar.copy(ctxT[:, h, c0:c0 + BC], ps_c)

                    # ---- normalize ctx^T for this block ----
                    recR = sup.tile([HEADS, BC], R, tag="recip")
                    with nc.allow_low_precision(
                            reason="f32r-rounded softmax reciprocal"):
                        nc.vector.reciprocal(recR, ps_sums)
                    for h in range(HEADS):
                        ps_b = psbc_p.tile([128, BC], f32, tag="psbc")
                        nc.tensor.matmul(ps_b, lhsT=sbc_sb[:, h, :],
                                         rhs=recR, start=True, stop=True)
                        rbc = rbp.tile([128, BC], bf16, tag="rbc")
                        nc.scalar.copy(rbc, ps_b)
                        nc.vector.tensor_mul(ctxT[:, h, c0:c0 + BC],
                                             ctxT[:, h, c0:c0 + BC], rbc)

                # ---- out = ctx @ c_proj_w + b ----
                for f in range(4):
                    wp_sb = wpp.tile([128, HEADS, 512], bf16, tag="wp")
                    nc.sync.dma_start(out=wp_sb, in_=wpr[:, f])
                    for m in range(T // 128):
                        ps_o = psb.tile([128, 512], f32, tag="ps", name="ps_o")
                        for h in range(HEADS):
                            nc.tensor.matmul(
                                ps_o, lhsT=ctxT[:, h, m * 128:(m + 1) * 128],
                                rhs=wp_sb[:, h, :], start=(h == 0),
                                stop=(h == HEADS - 1))
                        osb = outp.tile([128, 512], f32, tag="osb")
                        nc.vector.tensor_add(osb, ps_o,
                                             pbias[:, f * 512:(f + 1) * 512])
                        nc.sync.dma_start(
                            out=out[m * 128:(m + 1) * 128,
                                    f * 512:(f + 1) * 512],
                            in_=osb)
    nc.compile()
    return nc


_NC = {}


def _get_nc(reps=1):
    if reps not in _NC:
        _NC[reps] = _build_nc(reps)
    return _NC[reps]


def make_in_maps(x, c_attn_w, c_attn_b, c_proj_w, c_proj_b):
    import ml_dtypes

    bf = ml_dtypes.bfloat16
    x = np.asarray(x, np.float32)
    W = np.asarray(c_attn_w, np.float32)
    Wp = np.asarray(c_proj_w, np.float32)

    # waq[p, h, c, et, j] = W[et*128+p, c*E + h*128 + j]
    w4 = W.reshape(ET, 128, 3, HEADS, 128)  # [et, p, c, h, j]
    waq = np.ascontiguousarray(w4.transpose(1, 3, 2, 0, 4)).astype(bf)

    # cab[p, h*3+c] = b[c*E + h*128 + p], q part pre-scaled
    b_mod = np.asarray(c_attn_b, np.float32).copy()
    b_mod[:E] *= SCALE2
    b4 = b_mod.reshape(3, HEADS, 128)  # [c, h, p]
    cab = np.ascontiguousarray(b4.transpose(2, 1, 0).reshape(128, HEADS * 3))

    # wpr[p, f, h, j] = Wp[h*128+p, f*512+j]
    wp4 = Wp.reshape(HEADS, 128, 4, 512)  # [h, p, f, j]
    wpr = np.ascontiguousarray(wp4.transpose(1, 2, 0, 3)).astype(bf)

    # additive causal mask in [k, q] layout
    p = np.arange(128)[:, None]
    c = np.arange(BC)[None, :]
    mneg = np.stack([np.where(c >= kt * 128 + p, 0.0, NEG).astype(np.float32)
                     for kt in range(4)], axis=1)
    mneg = np.ascontiguousarray(mneg)

    ident = np.eye(128).astype(bf)
    # osel[p, h, j] = (j == h): col-sum of head h lands on psum partition h
    osel = np.broadcast_to(np.eye(HEADS, dtype=np.float32), (128, HEADS, HEADS))
    osel = np.ascontiguousarray(osel).astype(bf)
    # sbc[j, h, p] = (j == h): broadcasts recip[h] across 128 partitions
    sbc = np.ascontiguousarray(
        np.eye(HEADS, dtype=np.float32)[:, :, None]
        * np.ones((1, 1, 128), np.float32))

    xr = x.reshape(B * BLOCKS, BC, E)
    in_maps = []
    for core in range(NCORES):
        xs = xr[core * BPC:(core + 1) * BPC].reshape(T, E)
        xsT = np.ascontiguousarray(xs.T).astype(bf)
        in_maps.append({
            "xsT": xsT, "waq": waq, "cab": cab, "wpr": wpr,
            "cpb": np.ascontiguousarray(c_proj_b, np.float32),
            "mneg": mneg, "idnt": ident, "osel": osel, "sbc": sbc,
        })
    return in_maps


def kernel(x, c_attn_w, c_attn_b, c_proj_w, c_proj_b):
    from concourse import bass_utils

    nc = _get_nc()
    in_maps = make_in_maps(x, c_attn_w, c_attn_b, c_proj_w, c_proj_b)
    res = bass_utils.run_bass_kernel_spmd(nc, in_maps, core_ids=list(range(NCORES)))
    outs = [res.results[c]["out"] for c in range(NCORES)]
    full = np.concatenate(outs, axis=0).reshape(B, L, E).astype(np.float32)
    return full
